# revision 1
# baseline (speedup 1.0000x reference)
"""MiniRocket feature extractor on 8 Trainium2 NeuronCores (optimized).

Per core (4 batch rows), per (dilation, batch) unit:
  - one strided DMA builds xshift [72, 4096] fp32r in SBUF (9 dilated tap
    shifts x 8 channels of the zero-padded series),
  - TensorE computes resp = W^T @ xshift (contraction 72 = channel x tap;
    fp32r streams at full rate),
  - ScalarE drains PSUM fp32 -> SBUF int16 with a per-kernel scale chosen so
    all four bias thresholds sit inside +/-32000 (uniform grid => comparison
    flips only within max|bias|/32000 of a threshold; conversion is exact
    round+saturate),
  - VectorE runs fused compare+count (is_gt + add-reduce) per scaled bias over
    the full range, plus one strided pass covering both pad edges at once
    (one (d, f) pair's full count runs on ScalarE as sum-of-sign to balance
    engine load),
  - GpSimd applies the trim-parity/normalization finalize and the output DMA
    (keeps the sync DMA queue free for input prefetch).
"""

import numpy as np
from contextlib import ExitStack

import concourse.bass as bass
import concourse.mybir as mybir
import concourse.tile as tile
from concourse.ap import AP
from concourse.bass_utils import run_bass_kernel_spmd

DILATIONS = (1, 2, 4, 8, 16, 32)
ND = 6
K = 84
KS = 9
C = 8
L = 4096
F = 4
B = 32
N_CORES = 8
B_LOC = 4
PADMAX = 128
LP = L + 2 * PADMAX
KP = 96
NFEAT = ND * K * F
# (dilation, f) full-range counts computed on ScalarE instead of VectorE
ACT_OFFLOAD = frozenset({(5, 0)})

F32 = mybir.dt.float32
F16 = mybir.dt.float16
F32R = mybir.dt.float32r
I16 = mybir.dt.int16


def _split_excess_waits(nc, max_waits=1):
    """This walrus build allows only one sync-wait per instruction; hoist
    extra waits onto preceding NOPs of the same engine."""
    n = 0
    for f in nc.m.functions:
        for bb in f.blocks:
            insts = bb.instructions
            if not any(
                i.sync_info and i.sync_info.on_wait and len(i.sync_info.on_wait) > max_waits
                for i in insts
            ):
                continue
            out = []
            for inst in insts:
                si = inst.sync_info
                waits = list(si.on_wait) if si and si.on_wait else []
                if len(waits) > max_waits:
                    for w in waits[:-max_waits]:
                        nop = mybir.InstNoOp(name=f"syncfix-{n}", ins=[], outs=[])
                        n += 1
                        nop.engine = inst.engine
                        nop.sync_info = mybir.SyncInfo(on_wait=[w], on_update=[])
                        out.append(nop)
                    inst.sync_info = mybir.SyncInfo(
                        on_wait=waits[-max_waits:],
                        on_update=list(si.on_update or []),
                    )
                out.append(inst)
            bb.instructions = out


def _build_nc():
    nc = bass.Bass()
    xprep = nc.declare_dram_parameter("xprep", [B_LOC, C, LP], F32R, isOutput=False)
    wstack = nc.declare_dram_parameter("wstack", [ND, 72, KP], F32R, isOutput=False)
    cpack = nc.declare_dram_parameter("cpack", [KP, 126], F32, isOutput=False)
    out = nc.declare_dram_parameter("out", [B_LOC, NFEAT], F32, isOutput=True)

    alu = mybir.AluOpType
    NCC = ND * F

    with tile.TileContext(nc) as tc, ExitStack() as ctx:
        cpool = ctx.enter_context(tc.tile_pool(name="const", bufs=1))
        xsh_pool = ctx.enter_context(tc.tile_pool(name="xsh", bufs=8))
        psum_pool = ctx.enter_context(tc.tile_pool(name="psum", bufs=2, space="PSUM"))
        resp_pool = ctx.enter_context(tc.tile_pool(name="resp", bufs=4))
        trash_pool = ctx.enter_context(tc.tile_pool(name="trash", bufs=1))
        acc_pool = ctx.enter_context(tc.tile_pool(name="acc", bufs=8))
        feat_pool = ctx.enter_context(tc.tile_pool(name="feat", bufs=12))

        # first two input loads go out before anything else on the queue
        early_xsh = {}
        for b0 in range(2):
            xsh = xsh_pool.tile([72, L], F32R)
            _i = nc.sync.dma_start(
                xsh[:],
                AP(xprep, b0 * C * LP + (PADMAX - 4), [[1, KS], [LP, C], [1, L]]),
            )
            _i.ins.bass_priority = 2000  # input data first on the queue
            early_xsh[b0] = xsh

        w_t = cpool.tile([72, ND * KP], F32R)
        _iw = nc.sync.dma_start(w_t[:], AP(wstack, 0, [[KP, 72], [72 * KP, ND], [1, KP]]))
        _iw.ins.bass_priority = 1500

        cp_t = cpool.tile([KP, 126], F32, tag="cp_t")
        _ic = nc.sync.dma_start(cp_t[:], AP(cpack, 0, [[126, KP], [1, 126]]))
        _ic.ins.bass_priority = 1000
        bias_t = cp_t[:, 0:24]
        a_base = cp_t[:, 24:48]
        b_base = cp_t[:, 48:72]
        mean_t = cp_t[:, 72:96]
        std_t = cp_t[:, 96:120]
        ss_t = cp_t[:, 120:126]

        rstd_t = cpool.tile([KP, NCC], F32, tag="rstd_t")
        nc.vector.reciprocal(rstd_t[:], std_t)
        a_t = cpool.tile([KP, NCC], F32, tag="a_t")
        nc.gpsimd.tensor_mul(a_t[:], a_base, rstd_t[:])
        b_t = cpool.tile([KP, NCC], F32, tag="b_t")
        nc.gpsimd.tensor_mul(b_t[:], b_base, rstd_t[:])
        c_t = cpool.tile([KP, NCC], F32, tag="c_t")
        nc.gpsimd.tensor_mul(c_t[:], mean_t, rstd_t[:])
        nbias_t = cpool.tile([KP, NCC], F32, tag="nbias_t")
        nc.gpsimd.tensor_scalar_mul(nbias_t[:], bias_t, -1.0)

        trash = trash_pool.tile([KP, L], I16)
        trash2 = trash_pool.tile([KP, L], I16, tag="trash2")

        for di, d in enumerate(DILATIONS):
            pad = 4 * d
            w_d = w_t[:, di * KP : (di + 1) * KP]
            for b in range(B_LOC):
                if di == 0 and b in early_xsh:
                    xsh = early_xsh[b]
                else:
                    xsh = xsh_pool.tile([72, L], F32R)
                    nc.sync.dma_start(
                        xsh[:],
                        AP(xprep, b * C * LP + (PADMAX - pad), [[d, KS], [LP, C], [1, L]]),
                    )

                resp16 = resp_pool.tile([KP, L], I16)
                for h in range(2):
                    ps = psum_pool.tile([KP, 2048], F32)
                    for n in range(4):
                        nc.tensor.matmul(
                            ps[:, n * 512 : (n + 1) * 512],
                            w_d,
                            xsh[:, h * 2048 + n * 512 : h * 2048 + (n + 1) * 512],
                            start=True,
                            stop=True,
                        )
                    nc.scalar.activation(
                        resp16[:, h * 2048 : (h + 1) * 2048], ps[:],
                        mybir.ActivationFunctionType.Copy,
                        scale=ss_t[:, di : di + 1],
                    )

                acc = acc_pool.tile([KP, 2 * F], F32)
                col0 = di * F
                pstep = resp16[:].ap[0][0]
                tstep = trash[:].ap[0][0]
                for f in range(F):
                    b_ap = bias_t[:, col0 + f : col0 + f + 1]
                    if (di, f) in ACT_OFFLOAD:
                        # full count on ScalarE: sum of sign(resp - bias);
                        # finalize coefs are adjusted host-side
                        nc.scalar.activation(
                            trash2[:], resp16[:],
                            mybir.ActivationFunctionType.Sign,
                            bias=nbias_t[:, col0 + f : col0 + f + 1],
                            accum_out=acc[:, f : f + 1],
                        )
                    else:
                        nc.vector.tensor_scalar(
                            trash[:], resp16[:], b_ap, None, alu.is_gt, alu.add,
                            accum_out=acc[:, f : f + 1],
                        )
                    # both pad edges in one strided op: free dims [2, pad]
                    ein = AP(
                        resp16[:].tensor, resp16[:].offset,
                        [[pstep, KP], [L - pad, 2], [1, pad]],
                    )
                    eout = AP(
                        trash[:].tensor, trash[:].offset,
                        [[tstep, KP], [L - pad, 2], [1, pad]],
                    )
                    nc.vector.tensor_scalar(
                        eout, ein, b_ap, None, alu.is_gt, alu.add,
                        accum_out=acc[:, F + f : F + f + 1],
                    )

                # finalize on GPSIMD: feat = full*A - edges*B - C
                a_d = a_t[:, col0 : col0 + F]
                b_d = b_t[:, col0 : col0 + F]
                c_d = c_t[:, col0 : col0 + F]
                u = feat_pool.tile([KP, F], F32)
                nc.gpsimd.tensor_mul(u[:], acc[:, 0:F], a_d)
                w2 = feat_pool.tile([KP, F], F32)
                nc.gpsimd.tensor_mul(w2[:], acc[:, F : 2 * F], b_d)
                ft = feat_pool.tile([KP, F], F32)
                nc.gpsimd.tensor_sub(ft[:], u[:], w2[:])
                fn = feat_pool.tile([KP, F], F32)
                nc.gpsimd.tensor_sub(fn[:], ft[:], c_d)

                dst = AP(out, b * NFEAT + di * K * F, [[F, K], [1, F]])
                nc.gpsimd.dma_start(dst, fn[0:K, :])

    _split_excess_waits(nc)
    return nc


_NC_CACHE = None


def _get_nc():
    global _NC_CACHE
    if _NC_CACHE is None:
        _NC_CACHE = _build_nc()
    return _NC_CACHE


LAST_RESULTS = None


def kernel(x, channel_masks, bias_matrices, feature_mean, feature_std):
    global LAST_RESULTS
    x = np.ascontiguousarray(np.asarray(x, dtype=np.float32))
    masks = np.asarray(channel_masks, dtype=np.float32)
    biasm = np.asarray(bias_matrices, dtype=np.float32)
    mean = np.asarray(feature_mean, dtype=np.float32)
    std = np.asarray(feature_std, dtype=np.float32)

    wstack = np.zeros((ND, 72, KP), np.float32)
    for di in range(ND):
        wt = -masks[di].T
        for j in range(KS):
            wstack[di, j * C : (j + 1) * C, :K] = wt
    # int16 drain: resp is stored as round(resp * s_dk); compare against
    # bias * s_dk. s_dk chosen so all four thresholds sit inside +/-32000.
    sscale = np.zeros((ND, KP), np.float32)
    maxb = np.maximum(np.abs(biasm).max(axis=-1), 1e-6)  # [ND, K]
    sscale[:, :K] = 32000.0 / maxb
    biases_pad = np.full((ND, KP, F), 32100.0, np.float32)
    biases_pad[:, :K, :] = biasm * sscale[:, :K, None]

    coef_a = np.zeros((ND, KP, F), np.float32)
    coef_b = np.zeros((ND, KP, F), np.float32)
    nmean = np.zeros((ND, KP, F), np.float32)
    nstd = np.ones((ND, KP, F), np.float32)
    nmean[:, :K, :] = mean.reshape(ND, K, F)
    nstd[:, :K, :] = std.reshape(ND, K, F)
    for di, d in enumerate(DILATIONS):
        pad = 4 * d
        lt = L - 2 * pad
        par = ((di + np.arange(K)) % 2 == 1).astype(np.float32)[:, None]
        coef_a[di, :K, :] = np.where(par > 0, 1.0 / lt, 1.0 / L)
        coef_b[di, :K, :] = np.where(par > 0, 1.0 / lt, 0.0)
    # ACT-offloaded (d, f) pairs accumulate sum(sign(resp-b)) = 2*count - L,
    # so halve A and shift the mean term: feat = sgn*(A/2) - e*B - (C - L*A/2)
    for (di, f) in ACT_OFFLOAD:
        coef_a[di, :, f] *= 0.5
        nmean[di, :, f] -= float(L) * coef_a[di, :, f]

    xt = np.ascontiguousarray(x.transpose(0, 2, 1))
    xp = np.zeros((B, C, LP), np.float32)
    xp[:, :, PADMAX : PADMAX + L] = xt

    cpk = np.zeros((KP, 126), np.float32)
    cpk[:, 0:24] = biases_pad.transpose(1, 0, 2).reshape(KP, 24)
    cpk[:, 24:48] = coef_a.transpose(1, 0, 2).reshape(KP, 24)
    cpk[:, 48:72] = coef_b.transpose(1, 0, 2).reshape(KP, 24)
    cpk[:, 72:96] = nmean.transpose(1, 0, 2).reshape(KP, 24)
    cpk[:, 96:120] = nstd.transpose(1, 0, 2).reshape(KP, 24)
    cpk[:, 120:126] = sscale.T

    nc = _get_nc()
    in_maps = []
    for core in range(N_CORES):
        in_maps.append(
            {
                "xprep": np.ascontiguousarray(xp[core * B_LOC : (core + 1) * B_LOC]),
                "wstack": wstack,
                "cpack": cpk,
            }
        )
    res = run_bass_kernel_spmd(nc, in_maps, list(range(N_CORES)))
    LAST_RESULTS = res
    out = np.concatenate([res.results[i]["out"] for i in range(N_CORES)], axis=0)
    return out.astype(np.float32)



# revision 11
# speedup vs baseline: 1.1817x; 1.1817x over previous
"""MiniRocket feature extractor on 8 Trainium2 NeuronCores (packed-partition
version).

Per core (4 batch rows). The 6 dilations x 84 kernels = 504 response rows per
batch row are packed into 4 tiles of 126 partitions (engine cost is
proportional to free-dim size only, so 84-row tiles waste 1/3 of every pass):

  T0 = d0 k0..83  | d1 k0..41      T1 = d1 k42..83 | d2 k0..83
  T2 = d3 k0..83  | d4 k0..41      T3 = d4 k42..83 | d5 k0..83

With this packing the global feature row index is simply 126*T + r, so the
output layout is preserved with no permutation.

Per packed tile:
  - two dilations' xshift stacks [72, 4096] (fp16 -> half the DMA bytes of
    fp32; conv error ~1e-3 abs on a resp scale of ~5) feed accumulating
    matmul pairs into one PSUM tile [126, 512]-chunk-wise,
  - ScalarE drains PSUM fp32 -> SBUF int16 with a per-row scale placing all
    four bias thresholds inside +/-32000,
  - the four full-range threshold counts run as fused compare+accumulate
    passes split across VectorE (4x int16 mode), ScalarE (sign-accumulate)
    and Pool (gpsimd) to balance engine busy time,
  - VectorE handles the small strided pad-edge counts,
  - Pool applies the finalize (feat = full*A - edges*B - C, normalization
    baked in host-side) and one batched output DMA per batch row.
"""

import numpy as np
from contextlib import ExitStack

import concourse.bass as bass
import concourse.mybir as mybir
import concourse.tile as tile
from concourse.ap import AP
from concourse.bass_utils import run_bass_kernel_spmd

DILATIONS = (1, 2, 4, 8, 16, 32)
ND = 6
K = 84
KS = 9
C = 8
L = 4096
F = 4
B = 32
N_CORES = 8
B_LOC = 4
PADMAX = 128
LP = L + 2 * PADMAX
NFEAT = ND * K * F
NP = 126           # packed partition rows per tile
NT = 4             # packed tiles per batch row

# (dil_idx, k_start, count, row_start) pairs per packed tile
GROUPS = (
    ((0, 0, 84, 0), (1, 0, 42, 84)),
    ((1, 42, 42, 0), (2, 0, 84, 42)),
    ((3, 0, 84, 0), (4, 0, 42, 84)),
    ((4, 42, 42, 0), (5, 0, 84, 42)),
)

# Engine for each full-range count pass, keyed (g = b*4 + T, f).
# Default VectorE; 'A' = ScalarE sign-accumulate (finalize coefs adjusted
# host-side per (b, T, f)). Pool's ALU has no compare ops and no reduce
# form, so counting is split between VectorE and ScalarE only.
ENG_FULL = {}
for _g in (2, 5, 8, 11, 15):
    ENG_FULL[(_g, 3)] = "A"

# (g, h) drain halves executed on VectorE instead of ScalarE (balance knob)
DVE_DRAIN = set()

# cpack column layout
CP_BIAS = 0            # [126, 16]  scaled biases, col = T*4 + f
CP_NBIAS = 16          # [126, 16]  negated scaled biases (ScalarE sign bias)
CP_A = 32              # [126, 64]  full-count coef, col = b*16 + T*4 + f
CP_C = 96              # [126, 64]  constant term, col = b*16 + T*4 + f
CP_B1 = 160            # [126, 16]  edge coef, group-0 rows (zero elsewhere)
CP_B2 = 176            # [126, 16]  edge coef, group-1 rows (zero elsewhere)
CP_SS = 192            # [126, 4]   drain scale, col = T
CP_COLS = 196

F32 = mybir.dt.float32
F16 = mybir.dt.float16
I16 = mybir.dt.int16


def _split_excess_waits(nc, max_waits=1):
    """This walrus build allows only one sync-wait per instruction; hoist
    extra waits onto preceding NOPs of the same engine."""
    n = 0
    for f in nc.m.functions:
        for bb in f.blocks:
            insts = bb.instructions
            if not any(
                i.sync_info and i.sync_info.on_wait and len(i.sync_info.on_wait) > max_waits
                for i in insts
            ):
                continue
            out = []
            for inst in insts:
                si = inst.sync_info
                waits = list(si.on_wait) if si and si.on_wait else []
                if len(waits) > max_waits:
                    for w in waits[:-max_waits]:
                        nop = mybir.InstNoOp(name=f"syncfix-{n}", ins=[], outs=[])
                        n += 1
                        nop.engine = inst.engine
                        nop.sync_info = mybir.SyncInfo(on_wait=[w], on_update=[])
                        out.append(nop)
                    inst.sync_info = mybir.SyncInfo(
                        on_wait=waits[-max_waits:],
                        on_update=list(si.on_update or []),
                    )
                out.append(inst)
            bb.instructions = out


def _build_nc():
    nc = bass.Bass()
    xprep = nc.declare_dram_parameter("xprep", [B_LOC, C, LP], F16, isOutput=False)
    wstack = nc.declare_dram_parameter("wstack", [2 * NT, 72, NP], F16, isOutput=False)
    cpack = nc.declare_dram_parameter("cpack", [128, CP_COLS], F32, isOutput=False)
    out = nc.declare_dram_parameter("out", [B_LOC, NFEAT], F32, isOutput=True)

    alu = mybir.AluOpType

    with tile.TileContext(nc) as tc, ExitStack() as ctx:
        cpool = ctx.enter_context(tc.tile_pool(name="const", bufs=1))
        xsh_pool = ctx.enter_context(tc.tile_pool(name="xsh", bufs=8))
        psum_pool = ctx.enter_context(tc.tile_pool(name="psum", bufs=2, space="PSUM"))
        resp_pool = ctx.enter_context(tc.tile_pool(name="resp", bufs=3))
        trash_pool = ctx.enter_context(tc.tile_pool(name="trash", bufs=1))
        acc_pool = ctx.enter_context(tc.tile_pool(name="acc", bufs=8))
        feat_pool = ctx.enter_context(tc.tile_pool(name="feat", bufs=12))

        # first two input loads go out before anything else on the queue
        xsh = {}
        order = []
        for b in range(B_LOC):
            for d in range(ND):
                order.append((b, d))
        for i, (b, d) in enumerate(order[:2]):
            t = xsh_pool.tile([72, L], F16)
            _i = nc.sync.dma_start(
                t[:],
                AP(xprep, b * C * LP + (PADMAX - 4 * DILATIONS[d]), [[DILATIONS[d], KS], [LP, C], [1, L]]),
            )
            _i.ins.bass_priority = 2000
            xsh[(b, d)] = t

        w_t = cpool.tile([72, 2 * NT * NP], F16)
        _iw = nc.sync.dma_start(w_t[:], AP(wstack, 0, [[NP, 72], [72 * NP, 2 * NT], [1, NP]]))
        _iw.ins.bass_priority = 1500

        cp_t = cpool.tile([128, CP_COLS], F32, tag="cp_t")
        _ic = nc.sync.dma_start(cp_t[:], AP(cpack, 0, [[CP_COLS, 128], [1, CP_COLS]]))
        _ic.ins.bass_priority = 1000

        # remaining input loads, in first-use order; bufs=8 turns this into a
        # sliding prefetch window on the in-order sync queue
        for (b, d) in order[2:]:
            t = xsh_pool.tile([72, L], F16)
            nc.sync.dma_start(
                t[:],
                AP(xprep, b * C * LP + (PADMAX - 4 * DILATIONS[d]), [[DILATIONS[d], KS], [LP, C], [1, L]]),
            )
            xsh[(b, d)] = t

        trash_v = trash_pool.tile([NP, L], I16)
        trash_a = trash_pool.tile([NP, L], I16, tag="trash_a")
        trash_p = trash_pool.tile([NP, L], I16, tag="trash_p")

        for b in range(B_LOC):
            featb = feat_pool.tile([NP, NT * F], F32)
            for T in range(NT):
                g = b * NT + T
                (dA, _, _, _), (dB, _, _, _) = GROUPS[T]
                xA = xsh[(b, dA)]
                xB = xsh[(b, dB)]
                wA = w_t[:, (2 * T) * NP : (2 * T + 1) * NP]
                wB = w_t[:, (2 * T + 1) * NP : (2 * T + 2) * NP]

                resp = resp_pool.tile([NP, L], I16)
                for h in range(2):
                    ps = psum_pool.tile([NP, 2048], F32)
                    for n in range(4):
                        c0 = h * 2048 + n * 512
                        nc.tensor.matmul(
                            ps[:, n * 512 : (n + 1) * 512], wA, xA[:, c0 : c0 + 512],
                            start=True, stop=False,
                        )
                        nc.tensor.matmul(
                            ps[:, n * 512 : (n + 1) * 512], wB, xB[:, c0 : c0 + 512],
                            start=False, stop=True,
                        )
                    if (g, h) in DVE_DRAIN:
                        nc.vector.tensor_scalar_mul(
                            resp[:, h * 2048 : (h + 1) * 2048], ps[:],
                            cp_t[0:NP, CP_SS + T : CP_SS + T + 1],
                        )
                    else:
                        nc.scalar.activation(
                            resp[:, h * 2048 : (h + 1) * 2048], ps[:],
                            mybir.ActivationFunctionType.Copy,
                            scale=cp_t[0:NP, CP_SS + T : CP_SS + T + 1],
                        )

                acc = acc_pool.tile([NP, 3 * F], F32)
                for f in range(4):
                    col = T * 4 + f
                    eng = ENG_FULL.get((g, f), "V")
                    if eng == "A":
                        nc.scalar.activation(
                            trash_a[:], resp[:],
                            mybir.ActivationFunctionType.Sign,
                            bias=cp_t[0:NP, CP_NBIAS + col : CP_NBIAS + col + 1],
                            accum_out=acc[:, f : f + 1],
                        )
                    elif eng == "P":
                        nc.gpsimd.tensor_scalar(
                            trash_p[:], resp[:],
                            cp_t[0:NP, CP_BIAS + col : CP_BIAS + col + 1], None,
                            alu.is_gt, alu.add,
                            accum_out=acc[:, f : f + 1],
                        )
                    else:
                        nc.vector.tensor_scalar(
                            trash_v[:], resp[:],
                            cp_t[0:NP, CP_BIAS + col : CP_BIAS + col + 1], None,
                            alu.is_gt, alu.add,
                            accum_out=acc[:, f : f + 1],
                        )

                # pad-edge counts: both edges in one strided op per (group, f).
                # Engine partition ranges must start at 0, so each group's op
                # covers all 126 rows into its own acc columns; the finalize
                # coefs (B1/B2) zero out the wrong-pad rows.
                pstep = resp[:].ap[0][0]
                tstep = trash_v[:].ap[0][0]
                for gi, (dg, _, _, _) in enumerate(GROUPS[T]):
                    pad = 4 * DILATIONS[dg]
                    for f in range(4):
                        col = T * 4 + f
                        ein = AP(
                            resp[:].tensor, resp[:].offset,
                            [[pstep, NP], [L - pad, 2], [1, pad]],
                        )
                        eout = AP(
                            trash_v[:].tensor, trash_v[:].offset,
                            [[tstep, NP], [L - pad, 2], [1, pad]],
                        )
                        nc.vector.tensor_scalar(
                            eout, ein,
                            cp_t[0:NP, CP_BIAS + col : CP_BIAS + col + 1], None,
                            alu.is_gt, alu.add,
                            accum_out=acc[:, (1 + gi) * F + f : (1 + gi) * F + f + 1],
                        )

                # finalize on Pool: feat = full*A - edgesA*B1 - edgesB*B2 - C
                ca = b * 16 + T * 4
                u = feat_pool.tile([NP, F], F32)
                nc.gpsimd.tensor_mul(u[:], acc[:, 0:F], cp_t[0:NP, CP_A + ca : CP_A + ca + F])
                w2 = feat_pool.tile([NP, F], F32)
                nc.gpsimd.tensor_mul(w2[:], acc[:, F : 2 * F], cp_t[0:NP, CP_B1 + T * 4 : CP_B1 + T * 4 + F])
                w3 = feat_pool.tile([NP, F], F32)
                nc.gpsimd.tensor_mul(w3[:], acc[:, 2 * F : 3 * F], cp_t[0:NP, CP_B2 + T * 4 : CP_B2 + T * 4 + F])
                ft = feat_pool.tile([NP, F], F32)
                nc.gpsimd.tensor_sub(ft[:], u[:], w2[:])
                ft2 = feat_pool.tile([NP, F], F32)
                nc.gpsimd.tensor_sub(ft2[:], ft[:], w3[:])
                nc.gpsimd.tensor_sub(
                    featb[:, T * F : (T + 1) * F], ft2[:],
                    cp_t[0:NP, CP_C + ca : CP_C + ca + F],
                )

            dst = AP(out, b * NFEAT, [[F, NP], [NP * F, NT], [1, F]])
            nc.gpsimd.dma_start(dst, featb[:])

    _split_excess_waits(nc)
    return nc


_NC_CACHE = None


def _get_nc():
    global _NC_CACHE
    if _NC_CACHE is None:
        _NC_CACHE = _build_nc()
    return _NC_CACHE


LAST_RESULTS = None


def kernel(x, channel_masks, bias_matrices, feature_mean, feature_std):
    global LAST_RESULTS
    x = np.ascontiguousarray(np.asarray(x, dtype=np.float32))
    masks = np.asarray(channel_masks, dtype=np.float32)
    biasm = np.asarray(bias_matrices, dtype=np.float32)
    mean = np.asarray(feature_mean, dtype=np.float32)
    std = np.asarray(feature_std, dtype=np.float32)

    # packed weight stacks: one [72, 126] matrix per (tile, group)
    wpack = np.zeros((2 * NT, 72, NP), np.float16)
    for T in range(NT):
        for gi, (d, k0, cnt, r0) in enumerate(GROUPS[T]):
            wt = -masks[d, k0 : k0 + cnt].T  # [C, cnt]
            for j in range(KS):
                wpack[2 * T + gi, j * C : (j + 1) * C, r0 : r0 + cnt] = wt

    # int16 drain: resp stored as round(resp * s_dk); compare against
    # bias * s_dk. s_dk chosen so all four thresholds sit inside +/-32000.
    maxb = np.maximum(np.abs(biasm).max(axis=-1), 1e-6)  # [ND, K]
    sscale = 32000.0 / maxb                              # [ND, K]
    bias_s = biasm * sscale[:, :, None]                  # [ND, K, F]

    # normalization-baked finalize coefs per (d, k, f)
    coef_a = np.zeros((ND, K, F), np.float32)
    coef_b = np.zeros((ND, K, F), np.float32)
    coef_c = np.zeros((ND, K, F), np.float32)
    meanr = mean.reshape(ND, K, F)
    stdr = std.reshape(ND, K, F)
    for di, d in enumerate(DILATIONS):
        pad = 4 * d
        lt = L - 2 * pad
        par = ((di + np.arange(K)) % 2 == 1).astype(np.float32)[:, None]
        A = np.where(par > 0, 1.0 / lt, 1.0 / L)
        Bc = np.where(par > 0, 1.0 / lt, 0.0)
        coef_a[di] = A / stdr[di]
        coef_b[di] = Bc / stdr[di]
        coef_c[di] = meanr[di] / stdr[di]

    # packed per-row tables, row r of tile T = (d, k) per GROUPS
    bias_pk = np.zeros((NP, NT, F), np.float32)
    ss_pk = np.ones((NP, NT), np.float32)
    a_pk = np.zeros((NP, NT, F), np.float32)
    b1_pk = np.zeros((NP, NT, F), np.float32)
    b2_pk = np.zeros((NP, NT, F), np.float32)
    c_pk = np.zeros((NP, NT, F), np.float32)
    for T in range(NT):
        for gi, (d, k0, cnt, r0) in enumerate(GROUPS[T]):
            sl = slice(r0, r0 + cnt)
            ks = slice(k0, k0 + cnt)
            bias_pk[sl, T] = bias_s[d, ks]
            ss_pk[sl, T] = sscale[d, ks]
            a_pk[sl, T] = coef_a[d, ks]
            (b1_pk if gi == 0 else b2_pk)[sl, T] = coef_b[d, ks]
            c_pk[sl, T] = coef_c[d, ks]

    # per-(b, T, f) A/C (ScalarE sign-counted passes accumulate
    # sum(sign(resp-b)) = 2*count - L: halve A, shift C)
    a_btf = np.broadcast_to(a_pk[:, None], (NP, B_LOC, NT, F)).copy()
    c_btf = np.broadcast_to(c_pk[:, None], (NP, B_LOC, NT, F)).copy()
    for (g, f), eng in ENG_FULL.items():
        if eng == "A":
            b_, T_ = divmod(g, NT)
            a_btf[:, b_, T_, f] *= 0.5
            c_btf[:, b_, T_, f] -= float(L) * a_btf[:, b_, T_, f]

    cpk = np.zeros((128, CP_COLS), np.float32)
    cpk[:NP, CP_BIAS : CP_BIAS + 16] = bias_pk.reshape(NP, 16)
    cpk[:NP, CP_NBIAS : CP_NBIAS + 16] = -bias_pk.reshape(NP, 16)
    cpk[:NP, CP_A : CP_A + 64] = a_btf.reshape(NP, 64)
    cpk[:NP, CP_C : CP_C + 64] = c_btf.reshape(NP, 64)
    cpk[:NP, CP_B1 : CP_B1 + 16] = b1_pk.reshape(NP, 16)
    cpk[:NP, CP_B2 : CP_B2 + 16] = b2_pk.reshape(NP, 16)
    cpk[:NP, CP_SS : CP_SS + 4] = ss_pk

    xt = np.ascontiguousarray(x.transpose(0, 2, 1))
    xp = np.zeros((B, C, LP), np.float16)
    xp[:, :, PADMAX : PADMAX + L] = xt.astype(np.float16)

    nc = _get_nc()
    in_maps = []
    for core in range(N_CORES):
        in_maps.append(
            {
                "xprep": np.ascontiguousarray(xp[core * B_LOC : (core + 1) * B_LOC]),
                "wstack": wpack,
                "cpack": cpk,
            }
        )
    res = run_bass_kernel_spmd(nc, in_maps, list(range(N_CORES)))
    LAST_RESULTS = res
    out = np.concatenate([res.results[i]["out"] for i in range(N_CORES)], axis=0)
    return out.astype(np.float32)
